# revision 1
# baseline (speedup 1.0000x reference)
"""Trainium2 Bass kernel for a 2-layer LLaMA-style decoder with per-layer
memory K/V prefix (tokenmix2 Decoder), tensor-parallel over 8 NeuronCores.

Sharding: heads (32 -> 4/core), FFN intermediate (8192 -> 1024/core),
vocab (8192 -> 1024/core).  Two AllReduces per layer (attention out,
FFN out), each split into two sequence-chunk collectives for overlap.

Layouts: activations are kept transposed (xT [D, S]) so every matmul
output feeds the next without transposes.  Attention computes
scoresT [t, s] per head; softmax runs without max-subtraction (scores
are ~N(0,1) after the 1/sqrt(Dh) scale) with the normalization applied
on the oT psum evacuation.  Matmul compute in bf16, residual stream and
psum accumulation in fp32.
"""
import sys

sys.path.insert(0, "/opt/trn_rl_repo")

import numpy as np
import ml_dtypes

import concourse.bass as bass
import concourse.mybir as mybir
import concourse.tile as tile
from concourse import bacc
from concourse.bass_utils import run_bass_kernel_spmd

BF = np.float16

# model dims
L, D, H, DH, F, V = 2, 4096, 32, 128, 8192, 8192
B, S, M = 1, 1024, 512
T = M + S                      # 1536 total key positions
EPS = 1e-5
ROPE_BASE = 10000.0
SCALE = float(DH) ** -0.5

# per-core shards
NCORES = 8
HL = H // NCORES               # 4 local heads
DL = HL * DH                   # 512 local head dims
FL = F // NCORES               # 1024 local ffn
VL = V // NCORES               # 1024 local vocab
C = D // 128                   # 32 contraction tiles
NTT = T // 128                 # 12 key tiles
NMT = M // 128                 # 4 memory key tiles
NST = S // 128                 # 8 query tiles
NCH = 2                        # sequence chunks (512 each)
SC = S // NCH                  # 512

dt = mybir.dt
AF = mybir.ActivationFunctionType
ALU = mybir.AluOpType

NEG = -60000.0


def build_module():
    nc = bacc.Bacc("TRN2", target_bir_lowering=False, debug=False,
                   num_devices=NCORES)

    # const APs for activation scale/bias floats
    for v in (EPS, SCALE, 1.0 / D):
        t = nc.alloc_sbuf_tensor(f"cst_{v}", [128, 1], dt.float32)
        nc.gpsimd.memset(t.ap(), v)
        nc.const_aps.aps[(dt.float32, v)] = t.ap()

    # ---- kernel I/O ----
    h0T = nc.dram_tensor("h0T", [D, S], dt.float16, kind="ExternalInput")
    memT = nc.dram_tensor("memT", [L, 128, C, M], dt.float16, kind="ExternalInput")
    wqkvT = nc.dram_tensor("wqkvT", [L, 3, 2, 128, C, 256], dt.float16, kind="ExternalInput")
    wmT = nc.dram_tensor("wmT", [L, 2, 2, 128, C, 256], dt.float16, kind="ExternalInput")
    woT = nc.dram_tensor("woT", [L, 2, 128, HL, 2048], dt.float16, kind="ExternalInput")
    wguT = nc.dram_tensor("wguT", [L, 2, 8, 128, C, 128], dt.float16, kind="ExternalInput")
    wdT = nc.dram_tensor("wdT", [L, 4, 128, 8, 1024], dt.float16, kind="ExternalInput")
    lmT = nc.dram_tensor("lmT", [4, 128, C, 256], dt.float16, kind="ExternalInput")
    qcs = nc.dram_tensor("qcs", [128, 2, S], dt.float16, kind="ExternalInput")
    kcs = nc.dram_tensor("kcs", [128, 2, T], dt.float16, kind="ExternalInput")
    rmat_i = nc.dram_tensor("rmat", [128, 128], dt.float16, kind="ExternalInput")
    tmask = nc.dram_tensor("tmask", [128, 896], dt.float16, kind="ExternalInput")
    lnw = nc.dram_tensor("lnw", [128, 5, C], dt.float32, kind="ExternalInput")
    logitsT = nc.dram_tensor("logitsT", [VL, S], dt.float32, kind="ExternalOutput")

    with tile.TileContext(nc) as tc:
        with tc.tile_pool(name="sb", bufs=1) as sb, \
             tc.tile_pool(name="ps", bufs=1, space="PSUM") as ps, \
             tc.tile_pool(name="dr", bufs=1, space="DRAM") as dr:

            # ---- internal DRAM ----
            hdr = [dr.tile([D, S], dt.float16, tag=f"h{i}", bufs=1, name=f"h{i}")
                   for i in range(3)]           # h after resid 1..3
            arin = [[[dr.tile([D // 2, SC], dt.float16, tag=f"ai{i}{ch}{dh}",
                              bufs=1, name=f"ai{i}{ch}{dh}") for dh in range(2)]
                     for ch in range(NCH)] for i in range(2 * L)]
            arout = [[[dr.tile([D // 2, SC], dt.float16, tag=f"ao{i}{ch}{dh}",
                               bufs=1, addr_space="Shared",
                               name=f"ao{i}{ch}{dh}") for dh in range(2)]
                      for ch in range(NCH)] for i in range(2 * L)]
            mkTd = [dr.tile([128, HL, M], dt.float16, tag=f"mk{l}", bufs=1,
                            name=f"mk{l}") for l in range(L)]
            mvd = [dr.tile([128, HL, NMT, DH], dt.float16, tag=f"mv{l}", bufs=1,
                           name=f"mv{l}") for l in range(L)]

            # ---- global constants in SBUF ----
            qc = sb.tile([128, 2, S], dt.float16, tag="qc", bufs=1, name="qc")
            nc.sync.dma_start(qc[:], qcs[:])
            kc = sb.tile([128, 2, T], dt.float16, tag="kc", bufs=1, name="kc")
            nc.sync.dma_start(kc[:], kcs[:])
            rmat = sb.tile([128, 128], dt.float16, tag="rm", bufs=1, name="rmat")
            nc.sync.dma_start(rmat[:], rmat_i[:])
            mask = sb.tile([128, 896], dt.float16, tag="msk", bufs=1, name="mask")
            nc.sync.dma_start(mask[:], tmask[:])
            lns = sb.tile([128, 5, C], dt.float32, tag="ln", bufs=1, name="lns")
            nc.sync.dma_start(lns[:], lnw[:])
            ones_bf = sb.tile([128, 1], dt.float16, tag="o1", bufs=1, name="ones_bf")
            nc.vector.memset(ones_bf[:], 1.0)
            ones_row = sb.tile([1, 128], dt.float16, tag="o2", bufs=1, name="ones_row")
            nc.vector.memset(ones_row[:], 1.0)
            inv64_row = sb.tile([1, 128], dt.float16, tag="o3", bufs=1, name="inv64_row")
            nc.vector.memset(inv64_row[:], 1.0 / 64.0)

            def mm_ps(name):
                return ps.tile([128, 512], dt.float32, tag="mm", bufs=5, name=name)

            def aux_ps(name):
                return ps.tile([1, 512], dt.float32, tag="aux", bufs=1, name=name)

            def evf(name):
                return sb.tile([128, 512], dt.float32, tag="evf", bufs=1, name=name)

            def evh(name):
                return sb.tile([128, 512], dt.float16, tag="evh", bufs=2, name=name)

            def rope_apply(raw_ps, cos_ap, sin_ap, out_ap):
                """raw_ps: [128,512] psum f32 (pre-rope head tile, d on part).
                Writes rope'd bf16 into out_ap."""
                raw_bf = sb.tile([128, 512], dt.float16, tag="rraw", bufs=2,
                                 name="raw_bf")
                nc.vector.tensor_copy(raw_bf[:], raw_ps[:])
                r_ps = mm_ps("r_ps")
                nc.tensor.matmul(r_ps[:], rmat[:], raw_bf[:], start=True, stop=True)
                m1 = sb.tile([128, 512], dt.float16, tag="rt", bufs=2, name="m1")
                nc.vector.tensor_tensor(m1[:], raw_bf[:], cos_ap, ALU.mult)
                m2 = sb.tile([128, 512], dt.float16, tag="rt2", bufs=2, name="m2")
                nc.vector.tensor_tensor(m2[:], r_ps[:], sin_ap, ALU.mult)
                nc.vector.tensor_tensor(out_ap, m1[:], m2[:], ALU.add)

            # =========================================================
            # pre-phase: memory K/V projections for both layers -> DRAM
            # =========================================================
            for l in range(L):
                mem_sb = sb.tile([128, C, M], dt.float16, tag="xb", bufs=2,
                                 name=f"mem{l}")
                nc.sync.dma_start(mem_sb[:], memT[l])
                # mk: for each local head tile d -> [128, 512] then rope
                for half in range(2):
                    wmk = sb.tile([128, C, 256], dt.float16, tag="wp", bufs=2,
                                  name=f"wmk{l}{half}")
                    nc.sync.dma_start(wmk[:], wmT[l, 0, half])
                    for dd in range(2):
                        d = 2 * half + dd
                        acc = mm_ps(f"mk{l}{d}")
                        for c in range(C):
                            nc.tensor.matmul(acc[:], wmk[:, c, 128 * dd:128 * (dd + 1)],
                                             mem_sb[:, c, :], start=(c == 0),
                                             stop=(c == C - 1))
                        mko = sb.tile([128, 512], dt.float16, tag="pt", bufs=3,
                                      name="mko")
                        rope_apply(acc, kc[:, 0, :M], kc[:, 1, :M], mko[:])
                        nc.sync.dma_start(mkTd[l][:, d, :], mko[:])
                # mv: natural layout [m, d]
                for half in range(2):
                    wmv = sb.tile([128, C, 256], dt.float16, tag="wp", bufs=2,
                                  name=f"wmv{l}{half}")
                    nc.sync.dma_start(wmv[:], wmT[l, 1, half])
                    for mt in range(NMT):
                        acc = mm_ps(f"mv{l}{half}{mt}")
                        for c in range(C):
                            nc.tensor.matmul(acc[:, :256],
                                             mem_sb[:, c, 128 * mt:128 * (mt + 1)],
                                             wmv[:, c, :], start=(c == 0),
                                             stop=(c == C - 1))
                        mvo = sb.tile([128, 2, 128], dt.float16, tag="mvo", bufs=3,
                                      name="mvo")
                        nc.vector.tensor_copy(
                            mvo[:], acc[:, :256].rearrange("p (h d) -> p h d", d=128))
                        nc.sync.dma_start(mvd[l][:, 2 * half:2 * half + 2, mt, :],
                                          mvo[:])

            # =========================================================
            # rms pass: h_new = h_src (+ delta); write h_dst; xT = rms
            # =========================================================
            def rms_pass(h_src, delta, h_dst, ln_idx, xbufs, name):
                """h_src: DRAM [D, S] f32 AP; delta: list per chunk of DRAM
                [D, SC] or None; h_dst same form or None; xbufs: list per chunk
                of SBUF tiles [128, C, SC] bf16 (written in place)."""
                hv = h_src.rearrange("(c p) s -> p c s", p=128)
                for ch in range(NCH):
                    xb = xbufs[ch]
                    ssq = aux_ps(f"ssq_{name}{ch}")
                    for cq in range(C // 4):
                        csl = slice(4 * cq, 4 * cq + 4)
                        if delta is None:
                            # h goes straight into xb (fp16 -> fp16 DMA)
                            nc.sync.dma_start(
                                xb[:, csl, :],
                                hv[:, csl, SC * ch:SC * (ch + 1)])
                        else:
                            ht = sb.tile([128, 4, 512], dt.float16, tag="hl",
                                         bufs=2, name="ht")
                            nc.sync.dma_start(ht[:],
                                              hv[:, csl, SC * ch:SC * (ch + 1)])
                            dtl = sb.tile([128, 4, 512], dt.float16, tag="dl",
                                          bufs=2, name="dtl")
                            dh, dcq = cq // 4, cq % 4
                            nc.sync.dma_start(
                                dtl[:],
                                delta[ch][dh]
                                .rearrange("(c p) s -> p c s", p=128)
                                [:, slice(4 * dcq, 4 * dcq + 4), :])
                            for ci in range(4):
                                nc.vector.tensor_tensor(xb[:, 4 * cq + ci, :],
                                                        ht[:, ci, :],
                                                        dtl[:, ci, :], ALU.add)
                            if h_dst is not None:
                                nc.sync.dma_start(
                                    h_dst.rearrange("(c p) s -> p c s", p=128)
                                    [:, csl, SC * ch:SC * (ch + 1)],
                                    xb[:, csl, :])
                        for ci in range(4):
                            c = 4 * cq + ci
                            hsq = sb.tile([128, 512], dt.float16, tag="hsq",
                                          bufs=2, name="hsq")
                            nc.vector.tensor_tensor(hsq[:], xb[:, c, :],
                                                    xb[:, c, :], ALU.mult)
                            nc.tensor.matmul(ssq[:], ones_bf[:], hsq[:],
                                             start=(c == 0), stop=(c == C - 1))
                    # rsqrt row and broadcast
                    sq = sb.tile([1, 512], dt.float32, tag="row", bufs=2, name="sq")
                    nc.scalar.activation(sq[:], ssq[:], AF.Sqrt, bias=EPS,
                                         scale=1.0 / D)
                    rs = sb.tile([1, 512], dt.float16, tag="row2", bufs=2, name="rs")
                    with nc.allow_low_precision(reason="fp16 row for broadcast mm"):
                        nc.vector.reciprocal(rs[:], sq[:])
                    bc = ps.tile([128, 512], dt.float32, tag="bc", bufs=2, name="bc")
                    nc.tensor.matmul(bc[:], ones_row[:], rs[:], start=True,
                                     stop=True)
                    for c in range(C):
                        nc.vector.scalar_tensor_tensor(
                            xb[:, c, :], xb[:, c, :], lns[:, ln_idx, c:c + 1],
                            bc[:], ALU.mult, ALU.mult)

            # =========================================================
            # attention + Wo for one layer; xbufs hold xT
            # =========================================================
            def attn_phase(l, xbufs, ar_site):
                # KT per head / V built first (k, v, then per-head q + attn)
                KT = sb.tile([128, HL, T], dt.float16, tag="KT", bufs=1,
                             name=f"KT{l}")
                Vt = sb.tile([128, HL, NTT, DH], dt.float16, tag="V", bufs=1,
                             name=f"V{l}")
                nc.sync.dma_start(KT[:, :, :M], mkTd[l][:])
                nc.sync.dma_start(Vt[:, :, :NMT, :], mvd[l][:])
                # k/v projections, chunk-major so chunk 0 streams while
                # chunk 1's AR + rms still run
                for ch in range(NCH):
                    for half in range(2):
                        wk = sb.tile([128, C, 256], dt.float16, tag="wp", bufs=2,
                                     name=f"wk{l}{ch}{half}")
                        nc.sync.dma_start(wk[:], wqkvT[l, 1, half])
                        for dd in range(2):
                            d = 2 * half + dd
                            acc = mm_ps(f"k{l}{d}{ch}")
                            for c in range(C):
                                nc.tensor.matmul(
                                    acc[:], wk[:, c, 128 * dd:128 * (dd + 1)],
                                    xbufs[ch][:, c, :], start=(c == 0),
                                    stop=(c == C - 1))
                            rope_apply(acc, kc[:, 0, M + SC * ch:M + SC * (ch + 1)],
                                       kc[:, 1, M + SC * ch:M + SC * (ch + 1)],
                                       KT[:, d, M + SC * ch:M + SC * (ch + 1)])
                    for half in range(2):
                        wv = sb.tile([128, C, 256], dt.float16, tag="wp", bufs=2,
                                     name=f"wv{l}{ch}{half}")
                        nc.sync.dma_start(wv[:], wqkvT[l, 2, half])
                        for sti in range(4):
                            st = 4 * ch + sti
                            acc = mm_ps(f"v{l}{half}{st}")
                            for c in range(C):
                                nc.tensor.matmul(
                                    acc[:, :256],
                                    xbufs[ch][:, c, 128 * sti:128 * (sti + 1)],
                                    wv[:, c, :], start=(c == 0), stop=(c == C - 1))
                            nc.vector.tensor_copy(
                                Vt[:, 2 * half:2 * half + 2, NMT + st, :],
                                acc[:, :256].rearrange("p (h d) -> p h d", d=128))
                # per-head: q proj + attention
                oT = sb.tile([128, HL, S], dt.float16, tag="oT", bufs=1,
                             name=f"oT{l}")
                for half in range(2):
                    wqh = sb.tile([128, C, 256], dt.float16, tag="wp", bufs=2,
                                  name=f"wq{l}{half}")
                    nc.sync.dma_start(wqh[:], wqkvT[l, 0, half])
                    for hh in range(2):
                        h = 2 * half + hh
                        qT = sb.tile([128, S], dt.float16, tag="qT", bufs=2,
                                     name=f"qT{l}{h}")
                        for ch in range(NCH):
                            acc = mm_ps(f"q{l}{h}{ch}")
                            for c in range(C):
                                nc.tensor.matmul(
                                    acc[:], wqh[:, c, 128 * hh:128 * (hh + 1)],
                                    xbufs[ch][:, c, :], start=(c == 0),
                                    stop=(c == C - 1))
                            rope_apply(acc, qc[:, 0, SC * ch:SC * (ch + 1)],
                                       qc[:, 1, SC * ch:SC * (ch + 1)],
                                       qT[:, SC * ch:SC * (ch + 1)])
                        for sb_i in range(NCH):
                            ntt = NMT + 4 * (sb_i + 1)
                            o_ps = mm_ps(f"o{l}{h}{sb_i}")
                            s_ps = aux_ps(f"s{l}{h}{sb_i}")
                            for tt in range(ntt):
                                sc_ps = mm_ps(f"sc{l}{h}{sb_i}{tt}")
                                nc.tensor.matmul(sc_ps[:],
                                                 KT[:, h, 128 * tt:128 * (tt + 1)],
                                                 qT[:, SC * sb_i:SC * (sb_i + 1)],
                                                 start=True, stop=True)
                                dtile = tt - ntt + 4      # >= 0 -> diagonal tile
                                if dtile >= 0:
                                    off = 384 - 128 * dtile
                                    nc.vector.tensor_tensor(
                                        sc_ps[:], sc_ps[:],
                                        mask[:, off:off + 512], ALU.add)
                                pt = sb.tile([128, 512], dt.float16, tag="pt",
                                             bufs=3, name="pt")
                                nc.scalar.activation(pt[:], sc_ps[:], AF.Exp,
                                                     scale=SCALE)
                                nc.tensor.matmul(o_ps[:], Vt[:, h, tt, :], pt[:],
                                                 start=(tt == 0),
                                                 stop=(tt == ntt - 1))
                                nc.tensor.matmul(s_ps[:], ones_bf[:], pt[:],
                                                 start=(tt == 0),
                                                 stop=(tt == ntt - 1))
                            rrf = sb.tile([1, 512], dt.float32, tag="rowf", bufs=2,
                                          name="rrf")
                            nc.vector.reciprocal(rrf[:], s_ps[:])
                            rr = sb.tile([1, 512], dt.float16, tag="row2", bufs=2,
                                         name="rr")
                            with nc.allow_low_precision(reason="fp16 row for broadcast mm"):
                                nc.vector.tensor_scalar_mul(rr[:], rrf[:], 64.0)
                            bc = ps.tile([128, 512], dt.float32, tag="bc", bufs=2,
                                         name="bca")
                            nc.tensor.matmul(bc[:], inv64_row[:], rr[:],
                                             start=True, stop=True)
                            bcs = sb.tile([128, 512], dt.float32, tag="bcs",
                                          bufs=1, name="bcs")
                            nc.vector.tensor_copy(bcs[:], bc[:])
                            nc.vector.tensor_tensor(
                                oT[:, h, SC * sb_i:SC * (sb_i + 1)],
                                o_ps[:], bcs[:], ALU.mult)
                # Wo: out [Do, s] partial sums -> arin
                for ch in range(NCH):
                  for half in range(2):
                    wo = sb.tile([128, HL, 2048], dt.float16, tag="wp", bufs=2,
                                 name=f"wo{l}{ch}{half}")
                    nc.sync.dma_start(wo[:], woT[l, half])
                    for do in range(16):
                        if True:
                            acc = mm_ps(f"wo{l}{half}{do}{ch}")
                            for hh in range(HL):
                                nc.tensor.matmul(
                                    acc[:], wo[:, hh, 128 * do:128 * (do + 1)],
                                    oT[:, hh, SC * ch:SC * (ch + 1)],
                                    start=(hh == 0), stop=(hh == HL - 1))
                            ev = evh("woev")
                            nc.vector.tensor_copy(ev[:], acc[:])
                            nc.sync.dma_start(
                                arin[ar_site][ch][half]
                                .rearrange("(t p) s -> p t s", p=128)
                                [:, do, :], ev[:])
                    nc.gpsimd.collective_compute(
                        "AllReduce", ALU.add,
                        replica_groups=[list(range(NCORES))],
                        ins=[arin[ar_site][ch][half][:]],
                        outs=[arout[ar_site][ch][half][:]])

            # =========================================================
            # FFN for one layer: xbufs -> partial down-proj -> arin
            # =========================================================
            def ffn_phase(l, xbufs, ar_site):
                actT = sb.tile([128, FL // 128, S], dt.float16, tag="actT",
                               bufs=1, name=f"actT{l}")
                for ch in range(NCH):
                    for fe in range(FL // 128):
                        wg = sb.tile([128, C, 128], dt.float16, tag="wp", bufs=2,
                                     name=f"wg{l}{ch}{fe}")
                        nc.sync.dma_start(wg[:], wguT[l, 0, fe])
                        gs = sb.tile([128, 512], dt.float16, tag="gs", bufs=2,
                                     name="gs")
                        acc = mm_ps(f"g{l}{fe}{ch}")
                        for c in range(C):
                            nc.tensor.matmul(acc[:], wg[:, c, :],
                                             xbufs[ch][:, c, :], start=(c == 0),
                                             stop=(c == C - 1))
                        nc.scalar.activation(gs[:], acc[:], AF.Silu)
                        wu = sb.tile([128, C, 128], dt.float16, tag="wp", bufs=2,
                                     name=f"wu{l}{ch}{fe}")
                        nc.sync.dma_start(wu[:], wguT[l, 1, fe])
                        acc2 = mm_ps(f"u{l}{fe}{ch}")
                        for c in range(C):
                            nc.tensor.matmul(acc2[:], wu[:, c, :],
                                             xbufs[ch][:, c, :], start=(c == 0),
                                             stop=(c == C - 1))
                        nc.vector.tensor_tensor(
                            actT[:, fe, SC * ch:SC * (ch + 1)], acc2[:],
                            gs[:], ALU.mult)
                # down proj
                for ch in range(NCH):
                  for quarter in range(4):
                    wd = sb.tile([128, FL // 128, 1024], dt.float16, tag="wp",
                                 bufs=2, name=f"wd{l}{ch}{quarter}")
                    nc.sync.dma_start(wd[:], wdT[l, quarter])
                    for do in range(8):
                        if True:
                            acc = mm_ps(f"wd{l}{quarter}{do}{ch}")
                            for fc in range(FL // 128):
                                nc.tensor.matmul(
                                    acc[:], wd[:, fc, 128 * do:128 * (do + 1)],
                                    actT[:, fc, SC * ch:SC * (ch + 1)],
                                    start=(fc == 0), stop=(fc == FL // 128 - 1))
                            ev = evh("wdev")
                            nc.vector.tensor_copy(ev[:], acc[:])
                            nc.sync.dma_start(
                                arin[ar_site][ch][quarter // 2]
                                .rearrange("(t p) s -> p t s", p=128)
                                [:, 8 * (quarter % 2) + do, :], ev[:])
                    if quarter % 2 == 1:
                        nc.gpsimd.collective_compute(
                            "AllReduce", ALU.add,
                            replica_groups=[list(range(NCORES))],
                            ins=[arin[ar_site][ch][quarter // 2][:]],
                            outs=[arout[ar_site][ch][quarter // 2][:]])

            # =========================================================
            # main flow
            # =========================================================
            def xb_tiles(nm):
                return [sb.tile([128, C, SC], dt.float16, tag="xb", bufs=2,
                                name=f"{nm}{ch}") for ch in range(NCH)]

            # layer 0
            x0 = xb_tiles("x0")
            rms_pass(h0T[:], None, None, 0, x0, "r0")
            attn_phase(0, x0, 0)
            x1 = xb_tiles("x1")
            rms_pass(h0T[:], arout[0], hdr[0][:], 1, x1, "r1")
            ffn_phase(0, x1, 1)
            # layer 1
            x2 = xb_tiles("x2")
            rms_pass(hdr[0][:], arout[1], hdr[1][:], 2, x2, "r2")
            attn_phase(1, x2, 2)
            x3 = xb_tiles("x3")
            rms_pass(hdr[1][:], arout[2], hdr[2][:], 3, x3, "r3")
            ffn_phase(1, x3, 3)
            # final rms + lm head
            xf = xb_tiles("xf")
            rms_pass(hdr[2][:], arout[3], None, 4, xf, "rf")
            for ch in range(NCH):
              for vq in range(4):
                lm = sb.tile([128, C, 256], dt.float16, tag="wp", bufs=2,
                             name=f"lm{ch}{vq}")
                nc.sync.dma_start(lm[:], lmT[vq])
                for vv in range(2):
                    if True:
                        acc = mm_ps(f"lm{vq}{vv}{ch}")
                        for c in range(C):
                            nc.tensor.matmul(acc[:], lm[:, c, 128 * vv:128 * (vv + 1)],
                                             xf[ch][:, c, :], start=(c == 0),
                                             stop=(c == C - 1))
                        ev = evf("lmev")
                        nc.vector.tensor_copy(ev[:], acc[:])
                        nc.sync.dma_start(
                            logitsT[:].rearrange("(t p) s -> p t s", p=128)
                            [:, 2 * vq + vv, SC * ch:SC * (ch + 1)], ev[:])

    nc.finalize()
    return nc


_NC_CACHE = {}


def _get_module():
    if "nc" not in _NC_CACHE:
        _NC_CACHE["nc"] = build_module()
    return _NC_CACHE["nc"]


def _rope_tables():
    inv_freq = 1.0 / (ROPE_BASE ** (np.arange(0, DH, 2, dtype=np.float64) / DH))
    ang = np.arange(T, dtype=np.float64)[:, None] * inv_freq[None, :]
    emb = np.concatenate([ang, ang], axis=-1)          # [T, DH]
    return np.cos(emb).astype(np.float32), np.sin(emb).astype(np.float32)


def kernel(input_ids, memory, embed, Wq, Wk, Wv, Wo, Wg, Wu, Wd, Wmk, Wmv,
           ln1, ln2, normw, lm_head):
    input_ids = np.asarray(input_ids)
    f32 = np.float32
    memory = np.asarray(memory, f32)

    nc = _get_module()

    # host prep: embedding gather (pure data movement) + layout transforms
    h0 = np.asarray(embed, f32)[input_ids.reshape(-1)]          # [S, D]
    h0T = np.ascontiguousarray(h0.T).astype(BF)                 # [D, S] fp16

    cos, sin = _rope_tables()
    qcs = np.stack([cos[M:], sin[M:]]).transpose(2, 0, 1)       # [128, 2, S]
    kcs = np.stack([cos, sin]).transpose(2, 0, 1)               # [128, 2, T]

    rmat = np.zeros((128, 128), f32)
    for d in range(64):
        rmat[d + 64, d] = -1.0
        rmat[d, d + 64] = 1.0

    tmaskv = np.full((128, 896), NEG, f32)
    for t in range(128):
        tmaskv[t, 384 + t:] = 0.0

    def bf(x):
        return np.ascontiguousarray(x).astype(BF)

    def swz(wT, nsplit):
        """[Din, n] (Din = c*128) -> [nsplit, 128, c, n/nsplit]."""
        c = wT.shape[0] // 128
        n = wT.shape[1]
        w = wT.reshape(c, 128, n).transpose(1, 0, 2)            # [128, c, n]
        w = w.reshape(128, c, nsplit, n // nsplit).transpose(2, 0, 1, 3)
        return w

    memT = np.stack([swz(memory[l, 0].T, 1)[0] for l in range(L)])

    in_maps = []
    for i in range(NCORES):
        hs = slice(DL * i, DL * (i + 1))
        fs = slice(FL * i, FL * (i + 1))
        vs = slice(VL * i, VL * (i + 1))
        lnw = np.stack([np.asarray(ln1, f32)[0], np.asarray(ln2, f32)[0],
                        np.asarray(ln1, f32)[1], np.asarray(ln2, f32)[1],
                        np.asarray(normw, f32)])                # [5, D]
        in_maps.append({
            "h0T": h0T,
            "memT": bf(memT),
            "wqkvT": bf(np.stack([np.stack([swz(np.asarray(W, f32)[l][hs].T, 2)
                                            for W in (Wq, Wk, Wv)])
                                  for l in range(L)])),
            "wmT": bf(np.stack([np.stack([swz(np.asarray(W, f32)[l][hs].T, 2)
                                          for W in (Wmk, Wmv)])
                                for l in range(L)])),
            "woT": bf(np.stack([swz(np.asarray(Wo, f32)[l][:, hs].T, 2)
                                for l in range(L)])),
            "wguT": bf(np.stack([np.stack([swz(np.asarray(W, f32)[l][fs].T, 8)
                                           for W in (Wg, Wu)])
                                 for l in range(L)])),
            "wdT": bf(np.stack([swz(np.asarray(Wd, f32)[l][:, fs].T, 4)
                                for l in range(L)])),
            "lmT": bf(swz(np.asarray(lm_head, f32)[vs].T, 4)),
            "qcs": bf(qcs),
            "kcs": bf(kcs),
            "rmat": bf(rmat),
            "tmask": bf(tmaskv),
            "lnw": np.ascontiguousarray(
                lnw.reshape(5, C, 128).transpose(2, 0, 1)),     # [128, 5, C]
        })

    res = run_bass_kernel_spmd(nc, in_maps, core_ids=list(range(NCORES)))
    _NC_CACHE["last_results"] = res

    logits = np.empty((B, S, V), f32)
    for i in range(NCORES):
        logits[0, :, VL * i:VL * (i + 1)] = res.results[i]["logitsT"].T
    return logits



# revision 12
# speedup vs baseline: 1.0610x; 1.0610x over previous
"""Trainium2 Bass kernel for a 2-layer LLaMA-style decoder with per-layer
memory K/V prefix (tokenmix2 Decoder), tensor-parallel over 8 NeuronCores.

Sharding: heads (32 -> 4/core), FFN intermediate (8192 -> 1024/core),
vocab (8192 -> 1024/core).  One full-D AllReduce per sequence chunk per
residual site (4 sites x 2 chunks = 8 collectives of 4MB fp16).

Key scheduling idea: per-chunk interleaved emission so every AllReduce
is covered by independent compute in all engine queues:
  site k AR(ch0) <- covered by chunk-1 compute of the producing phase
  site k AR(ch1) <- covered by rms(ch0) + next phase's chunk-0 GEMMs
RMSNorm work rides the vector/gpsimd/scalar engines under the PE's GEMM
windows; ln weights are folded into the GEMM weights on the host.
"""
import sys

sys.path.insert(0, "/opt/trn_rl_repo")

import numpy as np
import ml_dtypes

import concourse.bass as bass
import concourse.mybir as mybir
import concourse.tile as tile
from concourse import bacc
from concourse.bass_utils import run_bass_kernel_spmd

BF = np.float16

# model dims
L, D, H, DH, F, V = 2, 4096, 32, 128, 8192, 8192
B, S, M = 1, 1024, 512
T = M + S                      # 1536 total key positions
EPS = 1e-5
ROPE_BASE = 10000.0
SCALE = float(DH) ** -0.5

# per-core shards
NCORES = 8
HL = H // NCORES               # 4 local heads
DL = HL * DH                   # 512 local head dims
FL = F // NCORES               # 1024 local ffn
VL = V // NCORES               # 1024 local vocab
C = D // 128                   # 32 contraction tiles
NTT = T // 128                 # 12 key tiles
NMT = M // 128                 # 4 memory key tiles
NST = S // 128                 # 8 query tiles
NCH = 2                        # sequence chunks (512 each)
SC = S // NCH                  # 512

dt = mybir.dt
AF = mybir.ActivationFunctionType
ALU = mybir.AluOpType

NEG = -60000.0


def build_module():
    nc = bacc.Bacc("TRN2", target_bir_lowering=False, debug=False,
                   num_devices=NCORES)

    # const APs for activation scale/bias floats
    for v in (EPS, SCALE, 1.0 / D):
        t = nc.alloc_sbuf_tensor(f"cst_{v}", [128, 1], dt.float32)
        nc.gpsimd.memset(t.ap(), v)
        nc.const_aps.aps[(dt.float32, v)] = t.ap()

    # ---- kernel I/O ----
    h0T = nc.dram_tensor("h0T", [D, S], dt.float16, kind="ExternalInput")
    memT = nc.dram_tensor("memT", [L, 2, 128, C, 256], dt.float16, kind="ExternalInput")
    wqkvT = nc.dram_tensor("wqkvT", [L, 3, 2, 128, C, 256], dt.float16, kind="ExternalInput")
    wmT = nc.dram_tensor("wmT", [L, 2, 2, 128, C, 256], dt.float16, kind="ExternalInput")
    woT = nc.dram_tensor("woT", [L, 2, 128, HL, 2048], dt.float16, kind="ExternalInput")
    wguT = nc.dram_tensor("wguT", [L, 2, 8, 128, C, 128], dt.float16, kind="ExternalInput")
    wdT = nc.dram_tensor("wdT", [L, 4, 128, 8, 1024], dt.float16, kind="ExternalInput")
    lmT = nc.dram_tensor("lmT", [4, 128, C, 256], dt.float16, kind="ExternalInput")
    kcs = nc.dram_tensor("kcs", [128, 2, T], dt.float16, kind="ExternalInput")
    rmat_i = nc.dram_tensor("rmat", [128, 128], dt.float16, kind="ExternalInput")
    tmask = nc.dram_tensor("tmask", [128, 896], dt.float16, kind="ExternalInput")
    logitsT = nc.dram_tensor("logitsT", [VL, S], dt.float32, kind="ExternalOutput")

    with tile.TileContext(nc) as tc:
        with tc.tile_pool(name="sb", bufs=1) as sb, \
             tc.tile_pool(name="ps", bufs=1, space="PSUM") as ps, \
             tc.tile_pool(name="dr", bufs=1, space="DRAM") as dr:

            # ---- internal DRAM ----
            hdr = [dr.tile([D, S], dt.float16, tag=f"h{i}", bufs=1, name=f"h{i}")
                   for i in range(3)]           # h after sites 0..2
            arin = [[dr.tile([D, SC], dt.float16, tag=f"ai{i}{ch}",
                             bufs=1, name=f"ai{i}{ch}") for ch in range(NCH)]
                    for i in range(2 * L)]
            arout = [[dr.tile([D, SC], dt.float16, tag=f"ao{i}{ch}",
                              bufs=1, addr_space="Shared",
                              name=f"ao{i}{ch}") for ch in range(NCH)]
                     for i in range(2 * L)]
            mkTd = [dr.tile([128, HL, M], dt.float16, tag=f"mk{l}", bufs=1,
                            name=f"mk{l}") for l in range(L)]
            mvd = [dr.tile([128, HL, NMT, DH], dt.float16, tag=f"mv{l}", bufs=1,
                           name=f"mv{l}") for l in range(L)]

            # ---- global constants in SBUF ----
            kc = sb.tile([128, 2, T], dt.float16, tag="kc", bufs=1, name="kc")
            nc.sync.dma_start(kc[:], kcs[:])
            rmat = sb.tile([128, 128], dt.float16, tag="rm", bufs=1, name="rmat")
            nc.sync.dma_start(rmat[:], rmat_i[:])
            mask = sb.tile([128, 896], dt.float16, tag="msk", bufs=1, name="mask")
            nc.sync.dma_start(mask[:], tmask[:])
            ones_bf = sb.tile([128, 1], dt.float16, tag="o1", bufs=1, name="ones_bf")
            nc.vector.memset(ones_bf[:], 1.0)
            ones_row = sb.tile([1, 128], dt.float16, tag="o2", bufs=1, name="ones_row")
            nc.vector.memset(ones_row[:], 1.0)
            inv64_row = sb.tile([1, 128], dt.float16, tag="o3", bufs=1, name="inv64_row")
            nc.vector.memset(inv64_row[:], 1.0 / 64.0)

            def mm_ps(name):
                return ps.tile([128, 512], dt.float32, tag="mm", bufs=5, name=name)

            def aux_ps(name, rows=1):
                return ps.tile([rows, 512], dt.float32, tag="aux", bufs=1,
                               name=name)

            def evf(name):
                return sb.tile([128, 512], dt.float32, tag="evf", bufs=1, name=name)

            def evh(name):
                return sb.tile([128, 512], dt.float16, tag="evh", bufs=2, name=name)

            def rope_apply(raw_ps, cos_ap, sin_ap, out_ap, w=512):
                """raw_ps: [128,w] psum f32 (pre-rope head tile, d on part).
                Writes rope'd fp16 into out_ap."""
                raw_bf = sb.tile([128, 512], dt.float16, tag="rraw", bufs=2,
                                 name="raw_bf")
                nc.vector.tensor_copy(raw_bf[:, :w], raw_ps)
                r_ps = mm_ps("r_ps")
                nc.tensor.matmul(r_ps[:, :w], rmat[:], raw_bf[:, :w],
                                 start=True, stop=True)
                m1 = sb.tile([128, 512], dt.float16, tag="rt", bufs=2, name="m1")
                nc.vector.tensor_tensor(m1[:, :w], raw_bf[:, :w], cos_ap, ALU.mult)
                m2 = sb.tile([128, 512], dt.float16, tag="rt2", bufs=2, name="m2")
                nc.vector.tensor_tensor(m2[:, :w], r_ps[:, :w], sin_ap, ALU.mult)
                nc.vector.tensor_tensor(out_ap, m1[:, :w], m2[:, :w], ALU.add)

            # =========================================================
            # memory K/V projections for one layer -> DRAM
            # (memory loaded in two M-halves of 256 to save SBUF)
            # =========================================================
            def memkv(l):
                for mh in range(2):
                    msl = slice(256 * mh, 256 * (mh + 1))
                    mem_h = sb.tile([128, C, 256], dt.float16, tag="xm", bufs=1,
                                    name=f"mem{l}{mh}")
                    nc.sync.dma_start(mem_h[:], memT[l, mh])
                    for half in range(2):
                        wmk = sb.tile([128, C, 256], dt.float16, tag="wp", bufs=2,
                                      name=f"wmk{l}{mh}{half}")
                        nc.sync.dma_start(wmk[:], wmT[l, 0, half])
                        for dd in range(2):
                            d = 2 * half + dd
                            acc = mm_ps(f"mk{l}{mh}{d}")
                            for c in range(C):
                                nc.tensor.matmul(
                                    acc[:, :256], wmk[:, c, 128 * dd:128 * (dd + 1)],
                                    mem_h[:, c, :], start=(c == 0),
                                    stop=(c == C - 1))
                            mko = sb.tile([128, 512], dt.float16, tag="pt", bufs=3,
                                          name="mko")
                            rope_apply(acc[:, :256], kc[:, 0, msl], kc[:, 1, msl],
                                       mko[:, :256], w=256)
                            nc.sync.dma_start(mkTd[l][:, d, msl], mko[:, :256])
                    for half in range(2):
                        wmv = sb.tile([128, C, 256], dt.float16, tag="wp", bufs=2,
                                      name=f"wmv{l}{mh}{half}")
                        nc.sync.dma_start(wmv[:], wmT[l, 1, half])
                        for mti in range(2):
                            mt = 2 * mh + mti
                            acc = mm_ps(f"mv{l}{mh}{half}{mt}")
                            for c in range(C):
                                nc.tensor.matmul(
                                    acc[:, :256],
                                    mem_h[:, c, 128 * mti:128 * (mti + 1)],
                                    wmv[:, c, :], start=(c == 0),
                                    stop=(c == C - 1))
                            mvo = sb.tile([128, 2, 128], dt.float16, tag="mvo",
                                          bufs=3, name="mvo")
                            nc.vector.tensor_copy(
                                mvo[:],
                                acc[:, :256].rearrange("p (h d) -> p h d", d=128))
                            nc.sync.dma_start(mvd[l][:, 2 * half:2 * half + 2, mt, :],
                                              mvo[:])

            # =========================================================
            # rms for ONE chunk: xb = rms(h_src + delta); h_dst written.
            # Squares split gpsimd/vector; sum-of-squares via one matmul.
            # =========================================================
            def rms_chunk(h_src, delta, h_dst, xb, ch, name):
                """h_src: DRAM [D,S] AP; delta: DRAM [D,SC] AP or None;
                h_dst: DRAM [D,S] AP or None; xb: SBUF tile [128,C,SC]."""
                hv = h_src.rearrange("(c p) s -> p c s", p=128)
                ssl = slice(SC * ch, SC * (ch + 1))
                sqacc = sb.tile([128, 512], dt.float16, tag="sqa", bufs=2,
                                name=f"sqa_{name}")
                for cq in range(C // 2):
                    csl = slice(2 * cq, 2 * cq + 2)
                    nc.sync.dma_start(xb[:, csl, :], hv[:, csl, ssl])
                    if delta is not None:
                        dtl = sb.tile([128, 2, 512], dt.float16, tag="dl",
                                      bufs=2, name="dtl")
                        nc.sync.dma_start(
                            dtl[:],
                            delta.rearrange("(c p) s -> p c s", p=128)[:, csl, :])
                        for ci in range(2):
                            nc.vector.tensor_tensor(xb[:, 2 * cq + ci, :],
                                                    xb[:, 2 * cq + ci, :],
                                                    dtl[:, ci, :], ALU.add)
                        if h_dst is not None:
                            nc.sync.dma_start(
                                h_dst.rearrange("(c p) s -> p c s", p=128)
                                [:, csl, ssl], xb[:, csl, :])
                    for ci in range(2):
                        c = 2 * cq + ci
                        hsq = sb.tile([128, 512], dt.float16, tag="hsq",
                                      bufs=2, name=f"hsq_{name}")
                        eng = nc.gpsimd if (c % 2 == 0) else nc.vector
                        eng.tensor_tensor(hsq[:], xb[:, c, :],
                                          xb[:, c, :], ALU.mult)
                        if c == 0:
                            nc.vector.tensor_copy(sqacc[:], hsq[:])
                        else:
                            nc.vector.tensor_tensor(sqacc[:], sqacc[:],
                                                    hsq[:], ALU.add)
                ssq = aux_ps(f"ssq_{name}")
                nc.tensor.matmul(ssq[:1, :], ones_bf[:], sqacc[:], start=True,
                                 stop=True)
                sq = sb.tile([1, 512], dt.float32, tag="row", bufs=1, name="sq")
                nc.scalar.activation(sq[:], ssq[:1, :], AF.Sqrt, bias=EPS,
                                     scale=1.0 / D)
                rs = sb.tile([1, 512], dt.float16, tag="row2", bufs=1, name="rs")
                with nc.allow_low_precision(reason="fp16 row for broadcast mm"):
                    nc.vector.reciprocal(rs[:], sq[:])
                bc = ps.tile([128, 512], dt.float32, tag="bc", bufs=2, name="bc")
                nc.tensor.matmul(bc[:], ones_row[:], rs[:], start=True, stop=True)
                bcs = sb.tile([128, 512], dt.float16, tag="bcs", bufs=2,
                              name=f"bcs_{name}")
                nc.vector.tensor_copy(bcs[:], bc[:])
                for c in range(C):
                    nc.vector.tensor_tensor(xb[:, c, :], xb[:, c, :], bcs[:],
                                            ALU.mult)

            # =========================================================
            # attention pieces (per layer, per chunk)
            # =========================================================
            def kv_chunk(l, ch, xb, KT, Vt):
                for half in range(2):
                    wk = sb.tile([128, C, 256], dt.float16, tag="wp", bufs=2,
                                 name=f"wk{l}{ch}{half}")
                    nc.sync.dma_start(wk[:], wqkvT[l, 1, half])
                    for dd in range(2):
                        d = 2 * half + dd
                        acc = mm_ps(f"k{l}{d}{ch}")
                        for c in range(C):
                            nc.tensor.matmul(
                                acc[:], wk[:, c, 128 * dd:128 * (dd + 1)],
                                xb[:, c, :], start=(c == 0), stop=(c == C - 1))
                        rope_apply(acc, kc[:, 0, M + SC * ch:M + SC * (ch + 1)],
                                   kc[:, 1, M + SC * ch:M + SC * (ch + 1)],
                                   KT[:, d, M + SC * ch:M + SC * (ch + 1)])
                for half in range(2):
                    wv = sb.tile([128, C, 256], dt.float16, tag="wp", bufs=2,
                                 name=f"wv{l}{ch}{half}")
                    nc.sync.dma_start(wv[:], wqkvT[l, 2, half])
                    for sti in range(4):
                        st = 4 * ch + sti
                        acc = mm_ps(f"v{l}{half}{st}")
                        for c in range(C):
                            nc.tensor.matmul(
                                acc[:, :256],
                                xb[:, c, 128 * sti:128 * (sti + 1)],
                                wv[:, c, :], start=(c == 0), stop=(c == C - 1))
                        nc.vector.tensor_copy(
                            Vt[:, 2 * half:2 * half + 2, NMT + st, :],
                            acc[:, :256].rearrange("p (h d) -> p h d", d=128))

            def q_chunk(l, ch, xb, qTc):
                """qTc: [128, HL, 512] rope'd queries for this chunk."""
                for half in range(2):
                    wqh = sb.tile([128, C, 256], dt.float16, tag="wp", bufs=2,
                                  name=f"wq{l}{ch}{half}")
                    nc.sync.dma_start(wqh[:], wqkvT[l, 0, half])
                    for hh in range(2):
                        h = 2 * half + hh
                        acc = mm_ps(f"q{l}{h}{ch}")
                        for c in range(C):
                            nc.tensor.matmul(
                                acc[:], wqh[:, c, 128 * hh:128 * (hh + 1)],
                                xb[:, c, :], start=(c == 0), stop=(c == C - 1))
                        rope_apply(acc, kc[:, 0, M + SC * ch:M + SC * (ch + 1)],
                                   kc[:, 1, M + SC * ch:M + SC * (ch + 1)],
                                   qTc[:, h, :])

            def scores_chunk(l, sb_i, qTc, KT, Vt, oTc):
                """softmax(qK)V for query chunk sb_i; writes oTc [128,HL,512]."""
                ntt = NMT + 4 * (sb_i + 1)
                for h in range(HL):
                    o_ps = mm_ps(f"o{l}{h}{sb_i}")
                    s_ps = aux_ps(f"s{l}{h}{sb_i}")
                    for tt in range(ntt):
                        sc_ps = mm_ps(f"sc{l}{h}{sb_i}{tt}")
                        nc.tensor.matmul(sc_ps[:],
                                         KT[:, h, 128 * tt:128 * (tt + 1)],
                                         qTc[:, h, :], start=True, stop=True)
                        dtile = tt - ntt + 4      # >= 0 -> diagonal tile
                        if dtile >= 0:
                            off = 384 - 128 * dtile
                            nc.vector.tensor_tensor(
                                sc_ps[:], sc_ps[:],
                                mask[:, off:off + 512], ALU.add)
                        pt = sb.tile([128, 512], dt.float16, tag="pt",
                                     bufs=3, name="pt")
                        nc.scalar.activation(pt[:], sc_ps[:], AF.Exp,
                                             scale=SCALE)
                        nc.tensor.matmul(o_ps[:], Vt[:, h, tt, :], pt[:],
                                         start=(tt == 0), stop=(tt == ntt - 1))
                        nc.tensor.matmul(s_ps[:], ones_bf[:], pt[:],
                                         start=(tt == 0), stop=(tt == ntt - 1))
                    rrf = sb.tile([1, 512], dt.float32, tag="rowf", bufs=1,
                                  name="rrf")
                    nc.vector.reciprocal(rrf[:], s_ps[:])
                    rr = sb.tile([1, 512], dt.float16, tag="row2", bufs=1,
                                 name="rr")
                    with nc.allow_low_precision(reason="fp16 row for broadcast mm"):
                        nc.vector.tensor_scalar_mul(rr[:], rrf[:], 64.0)
                    bc = ps.tile([128, 512], dt.float32, tag="bc", bufs=2,
                                 name="bca")
                    nc.tensor.matmul(bc[:], inv64_row[:], rr[:],
                                     start=True, stop=True)
                    bcs = sb.tile([128, 512], dt.float32, tag="bcs2",
                                  bufs=1, name="bcs2")
                    nc.vector.tensor_copy(bcs[:], bc[:])
                    nc.vector.tensor_tensor(oTc[:, h, :], o_ps[:], bcs[:],
                                            ALU.mult)

            def wo_chunk(l, ch, oTc, site):
                for half in range(2):
                    wo = sb.tile([128, HL, 2048], dt.float16, tag="wp", bufs=2,
                                 name=f"wo{l}{ch}{half}")
                    nc.sync.dma_start(wo[:], woT[l, half])
                    for do in range(16):
                        acc = mm_ps(f"wo{l}{half}{do}{ch}")
                        for hh in range(HL):
                            nc.tensor.matmul(
                                acc[:], wo[:, hh, 128 * do:128 * (do + 1)],
                                oTc[:, hh, :],
                                start=(hh == 0), stop=(hh == HL - 1))
                        ev = evh("woev")
                        nc.vector.tensor_copy(ev[:], acc[:])
                        nc.sync.dma_start(
                            arin[site][ch]
                            .rearrange("(c p) s -> p c s", p=128)
                            [:, 16 * half + do, :], ev[:])
                nc.gpsimd.collective_compute(
                    "AllReduce", ALU.add,
                    replica_groups=[list(range(NCORES))],
                    ins=[arin[site][ch][:]],
                    outs=[arout[site][ch][:]])

            # =========================================================
            # FFN for one chunk
            # =========================================================
            def ffn_chunk(l, ch, xb, site):
                actT = sb.tile([128, FL // 128, 512], dt.float16, tag="actT",
                               bufs=1, name=f"actT{l}{ch}")
                for fe in range(FL // 128):
                    wg = sb.tile([128, C, 128], dt.float16, tag="wp", bufs=2,
                                 name=f"wg{l}{ch}{fe}")
                    nc.sync.dma_start(wg[:], wguT[l, 0, fe])
                    gs = sb.tile([128, 512], dt.float16, tag="gs", bufs=2,
                                 name="gs")
                    acc = mm_ps(f"g{l}{fe}{ch}")
                    for c in range(C):
                        nc.tensor.matmul(acc[:], wg[:, c, :], xb[:, c, :],
                                         start=(c == 0), stop=(c == C - 1))
                    nc.scalar.activation(gs[:], acc[:], AF.Silu)
                    wu = sb.tile([128, C, 128], dt.float16, tag="wp", bufs=2,
                                 name=f"wu{l}{ch}{fe}")
                    nc.sync.dma_start(wu[:], wguT[l, 1, fe])
                    acc2 = mm_ps(f"u{l}{fe}{ch}")
                    for c in range(C):
                        nc.tensor.matmul(acc2[:], wu[:, c, :], xb[:, c, :],
                                         start=(c == 0), stop=(c == C - 1))
                    nc.vector.tensor_tensor(actT[:, fe, :], acc2[:], gs[:],
                                            ALU.mult)
                for quarter in range(4):
                    wd = sb.tile([128, FL // 128, 1024], dt.float16, tag="wp",
                                 bufs=2, name=f"wd{l}{ch}{quarter}")
                    nc.sync.dma_start(wd[:], wdT[l, quarter])
                    for do in range(8):
                        acc = mm_ps(f"wd{l}{quarter}{do}{ch}")
                        for fc in range(FL // 128):
                            nc.tensor.matmul(
                                acc[:], wd[:, fc, 128 * do:128 * (do + 1)],
                                actT[:, fc, :],
                                start=(fc == 0), stop=(fc == FL // 128 - 1))
                        ev = evh("wdev")
                        nc.vector.tensor_copy(ev[:], acc[:])
                        nc.sync.dma_start(
                            arin[site][ch]
                            .rearrange("(c p) s -> p c s", p=128)
                            [:, 8 * quarter + do, :], ev[:])
                nc.gpsimd.collective_compute(
                    "AllReduce", ALU.add,
                    replica_groups=[list(range(NCORES))],
                    ins=[arin[site][ch][:]],
                    outs=[arout[site][ch][:]])

            # =========================================================
            # LM head for one chunk
            # =========================================================
            def lm_chunk(ch, xb):
                for vq in range(4):
                    lm = sb.tile([128, C, 256], dt.float16, tag="wp", bufs=2,
                                 name=f"lm{ch}{vq}")
                    nc.sync.dma_start(lm[:], lmT[vq])
                    for vv in range(2):
                        acc = mm_ps(f"lm{vq}{vv}{ch}")
                        for c in range(C):
                            nc.tensor.matmul(
                                acc[:], lm[:, c, 128 * vv:128 * (vv + 1)],
                                xb[:, c, :], start=(c == 0), stop=(c == C - 1))
                        ev = evf("lmev")
                        nc.vector.tensor_copy(ev[:], acc[:])
                        nc.sync.dma_start(
                            logitsT[:].rearrange("(t p) s -> p t s", p=128)
                            [:, 2 * vq + vv, SC * ch:SC * (ch + 1)], ev[:])

            # =========================================================
            # main flow
            # =========================================================
            def xb_tile(nm):
                return sb.tile([128, C, SC], dt.float16, tag="xb", bufs=2,
                               name=nm)

            def kvt_tiles(l):
                KT = sb.tile([128, HL, T], dt.float16, tag="KT", bufs=1,
                             name=f"KT{l}")
                Vt = sb.tile([128, HL, NTT, DH], dt.float16, tag="V", bufs=1,
                             name=f"V{l}")
                nc.sync.dma_start(KT[:, :, :M], mkTd[l][:])
                nc.sync.dma_start(Vt[:, :, :NMT, :], mvd[l][:])
                return KT, Vt

            def qo_tiles(l, ch):
                qTc = sb.tile([128, HL, 512], dt.float16, tag="qT", bufs=2,
                              name=f"qT{l}{ch}")
                oTc = sb.tile([128, HL, 512], dt.float16, tag="oT", bufs=2,
                              name=f"oT{l}{ch}")
                return qTc, oTc

            # ---- prephase: memory K/V layer 0, initial rms ----
            memkv(0)
            x_a0 = [xb_tile(f"xa0{ch}") for ch in range(NCH)]
            rms_chunk(h0T[:], None, None, x_a0[0], 0, "ra0")
            rms_chunk(h0T[:], None, None, x_a0[1], 1, "ra1")

            # ---- attention layer 0 (site 0) ----
            KT0, Vt0 = kvt_tiles(0)
            q00, o00 = qo_tiles(0, 0)
            kv_chunk(0, 0, x_a0[0], KT0, Vt0)
            q_chunk(0, 0, x_a0[0], q00)
            scores_chunk(0, 0, q00, KT0, Vt0, o00)
            wo_chunk(0, 0, o00, 0)
            memkv(1)                       # filler PE work under AR[0][0]
            q01, o01 = qo_tiles(0, 1)
            kv_chunk(0, 1, x_a0[1], KT0, Vt0)
            q_chunk(0, 1, x_a0[1], q01)
            scores_chunk(0, 1, q01, KT0, Vt0, o01)
            wo_chunk(0, 1, o01, 0)

            # ---- FFN layer 0 (site 1) ----
            x_f0 = [xb_tile(f"xf0{ch}") for ch in range(NCH)]
            rms_chunk(h0T[:], arout[0][0][:], hdr[0][:], x_f0[0], 0, "rf00")
            ffn_chunk(0, 0, x_f0[0], 1)
            rms_chunk(h0T[:], arout[0][1][:], hdr[0][:], x_f0[1], 1, "rf01")
            ffn_chunk(0, 1, x_f0[1], 1)

            # ---- attention layer 1 (site 2) ----
            x_a1 = [xb_tile(f"xa1{ch}") for ch in range(NCH)]
            rms_chunk(hdr[0][:], arout[1][0][:], hdr[1][:], x_a1[0], 0, "ra10")
            KT1, Vt1 = kvt_tiles(1)
            q10, o10 = qo_tiles(1, 0)
            kv_chunk(1, 0, x_a1[0], KT1, Vt1)
            q_chunk(1, 0, x_a1[0], q10)
            scores_chunk(1, 0, q10, KT1, Vt1, o10)
            wo_chunk(1, 0, o10, 2)
            rms_chunk(hdr[0][:], arout[1][1][:], hdr[1][:], x_a1[1], 1, "ra11")
            q11, o11 = qo_tiles(1, 1)
            kv_chunk(1, 1, x_a1[1], KT1, Vt1)
            q_chunk(1, 1, x_a1[1], q11)
            scores_chunk(1, 1, q11, KT1, Vt1, o11)
            wo_chunk(1, 1, o11, 2)

            # ---- FFN layer 1 (site 3) ----
            x_f1 = [xb_tile(f"xf1{ch}") for ch in range(NCH)]
            rms_chunk(hdr[1][:], arout[2][0][:], hdr[2][:], x_f1[0], 0, "rf10")
            ffn_chunk(1, 0, x_f1[0], 3)
            rms_chunk(hdr[1][:], arout[2][1][:], hdr[2][:], x_f1[1], 1, "rf11")
            ffn_chunk(1, 1, x_f1[1], 3)

            # ---- final rms + LM head ----
            xf = [xb_tile(f"xl{ch}") for ch in range(NCH)]
            rms_chunk(hdr[2][:], arout[3][0][:], None, xf[0], 0, "rl0")
            lm_chunk(0, xf[0])
            rms_chunk(hdr[2][:], arout[3][1][:], None, xf[1], 1, "rl1")
            lm_chunk(1, xf[1])

    nc.finalize()
    return nc


_NC_CACHE = {}


def _get_module():
    if "nc" not in _NC_CACHE:
        _NC_CACHE["nc"] = build_module()
    return _NC_CACHE["nc"]


def _rope_tables():
    inv_freq = 1.0 / (ROPE_BASE ** (np.arange(0, DH, 2, dtype=np.float64) / DH))
    ang = np.arange(T, dtype=np.float64)[:, None] * inv_freq[None, :]
    emb = np.concatenate([ang, ang], axis=-1)          # [T, DH]
    return np.cos(emb).astype(np.float32), np.sin(emb).astype(np.float32)


def kernel(input_ids, memory, embed, Wq, Wk, Wv, Wo, Wg, Wu, Wd, Wmk, Wmv,
           ln1, ln2, normw, lm_head):
    input_ids = np.asarray(input_ids)
    f32 = np.float32
    memory = np.asarray(memory, f32)

    nc = _get_module()

    # host prep: embedding gather (pure data movement) + layout transforms
    h0 = np.asarray(embed, f32)[input_ids.reshape(-1)]          # [S, D]
    h0T = np.ascontiguousarray(h0.T).astype(BF)                 # [D, S] fp16

    cos, sin = _rope_tables()
    qcs = np.stack([cos[M:], sin[M:]]).transpose(2, 0, 1)       # [128, 2, S]
    kcs = np.stack([cos, sin]).transpose(2, 0, 1)               # [128, 2, T]

    rmat = np.zeros((128, 128), f32)
    for d in range(64):
        rmat[d + 64, d] = -1.0
        rmat[d, d + 64] = 1.0

    tmaskv = np.full((128, 896), NEG, f32)
    for t in range(128):
        tmaskv[t, 384 + t:] = 0.0

    def bf(x):
        return np.ascontiguousarray(x).astype(BF)

    def swz(wT, nsplit):
        """[Din, n] (Din = c*128) -> [nsplit, 128, c, n/nsplit]."""
        c = wT.shape[0] // 128
        n = wT.shape[1]
        w = wT.reshape(c, 128, n).transpose(1, 0, 2)            # [128, c, n]
        w = w.reshape(128, c, nsplit, n // nsplit).transpose(2, 0, 1, 3)
        return w

    memT = np.stack([swz(memory[l, 0].T, 2) for l in range(L)])  # [L,2,128,C,256]

    # fold RMSNorm weights into the GEMM weights (applied along d-in)
    ln1 = np.asarray(ln1, f32)
    ln2 = np.asarray(ln2, f32)
    normw = np.asarray(normw, f32)
    Wq_f = np.asarray(Wq, f32) * ln1[:, None, :]
    Wk_f = np.asarray(Wk, f32) * ln1[:, None, :]
    Wv_f = np.asarray(Wv, f32) * ln1[:, None, :]
    Wg_f = np.asarray(Wg, f32) * ln2[:, None, :]
    Wu_f = np.asarray(Wu, f32) * ln2[:, None, :]
    lm_f = np.asarray(lm_head, f32) * normw[None, :]

    in_maps = []
    for i in range(NCORES):
        hs = slice(DL * i, DL * (i + 1))
        fs = slice(FL * i, FL * (i + 1))
        vs = slice(VL * i, VL * (i + 1))
        in_maps.append({
            "h0T": h0T,
            "memT": bf(memT),
            "wqkvT": bf(np.stack([np.stack([swz(W[l][hs].T, 2)
                                            for W in (Wq_f, Wk_f, Wv_f)])
                                  for l in range(L)])),
            "wmT": bf(np.stack([np.stack([swz(np.asarray(W, f32)[l][hs].T, 2)
                                          for W in (Wmk, Wmv)])
                                for l in range(L)])),
            "woT": bf(np.stack([swz(np.asarray(Wo, f32)[l][:, hs].T, 2)
                                for l in range(L)])),
            "wguT": bf(np.stack([np.stack([swz(W[l][fs].T, 8)
                                           for W in (Wg_f, Wu_f)])
                                 for l in range(L)])),
            "wdT": bf(np.stack([swz(np.asarray(Wd, f32)[l][:, fs].T, 4)
                                for l in range(L)])),
            "lmT": bf(swz(lm_f[vs].T, 4)),
            "kcs": bf(kcs),
            "rmat": bf(rmat),
            "tmask": bf(tmaskv),
        })

    res = run_bass_kernel_spmd(nc, in_maps, core_ids=list(range(NCORES)))
    _NC_CACHE["last_results"] = res

    logits = np.empty((B, S, V), f32)
    for i in range(NCORES):
        logits[0, :, VL * i:VL * (i + 1)] = res.results[i]["logitsT"].T
    return logits


# revision 15
# speedup vs baseline: 1.0754x; 1.0136x over previous
"""Trainium2 Bass kernel for a 2-layer LLaMA-style decoder with per-layer
memory K/V prefix (tokenmix2 Decoder), tensor-parallel over 8 NeuronCores.

Sharding: heads (32 -> 4/core), FFN intermediate (8192 -> 1024/core),
vocab (8192 -> 1024/core).  One full-D AllReduce per sequence chunk per
residual site (4 sites x 2 chunks = 8 collectives of 4MB fp16).

Key scheduling idea: per-chunk interleaved emission so every AllReduce
is covered by independent compute in all engine queues:
  site k AR(ch0) <- covered by chunk-1 compute of the producing phase
  site k AR(ch1) <- covered by rms(ch0) + next phase's chunk-0 GEMMs
RMSNorm work rides the vector/gpsimd/scalar engines under the PE's GEMM
windows; ln weights are folded into the GEMM weights on the host.
"""
import sys

sys.path.insert(0, "/opt/trn_rl_repo")

import numpy as np
import ml_dtypes

import concourse.bass as bass
import concourse.mybir as mybir
import concourse.tile as tile
from concourse import bacc
from concourse.bass_utils import run_bass_kernel_spmd

BF = np.float16

# model dims
L, D, H, DH, F, V = 2, 4096, 32, 128, 8192, 8192
B, S, M = 1, 1024, 512
T = M + S                      # 1536 total key positions
EPS = 1e-5
ROPE_BASE = 10000.0
SCALE = float(DH) ** -0.5

# per-core shards
NCORES = 8
HL = H // NCORES               # 4 local heads
DL = HL * DH                   # 512 local head dims
FL = F // NCORES               # 1024 local ffn
VL = V // NCORES               # 1024 local vocab
C = D // 128                   # 32 contraction tiles
NTT = T // 128                 # 12 key tiles
NMT = M // 128                 # 4 memory key tiles
NST = S // 128                 # 8 query tiles
NCH = 2                        # sequence chunks (512 each)
SC = S // NCH                  # 512

dt = mybir.dt
AF = mybir.ActivationFunctionType
ALU = mybir.AluOpType

NEG = -60000.0


def build_module():
    nc = bacc.Bacc("TRN2", target_bir_lowering=False, debug=False,
                   num_devices=NCORES)

    # const APs for activation scale/bias floats
    for v in (EPS, SCALE, 1.0 / D):
        t = nc.alloc_sbuf_tensor(f"cst_{v}", [128, 1], dt.float32)
        nc.gpsimd.memset(t.ap(), v)
        nc.const_aps.aps[(dt.float32, v)] = t.ap()

    # ---- kernel I/O ----
    h0T = nc.dram_tensor("h0T", [D, S], dt.float16, kind="ExternalInput")
    memT = nc.dram_tensor("memT", [L, 2, 128, C, 256], dt.float16, kind="ExternalInput")
    wqkvT = nc.dram_tensor("wqkvT", [L, 3, 4, 128, C, 128], dt.float16, kind="ExternalInput")
    wmT = nc.dram_tensor("wmT", [L, 2, 4, 128, C, 128], dt.float16, kind="ExternalInput")
    woT = nc.dram_tensor("woT", [L, 4, 128, HL, 1024], dt.float16, kind="ExternalInput")
    wguT = nc.dram_tensor("wguT", [L, 2, 8, 128, C, 128], dt.float16, kind="ExternalInput")
    wdT = nc.dram_tensor("wdT", [L, 8, 128, 8, 512], dt.float16, kind="ExternalInput")
    lmT = nc.dram_tensor("lmT", [8, 128, C, 128], dt.float16, kind="ExternalInput")
    kcs = nc.dram_tensor("kcs", [128, 2, T], dt.float16, kind="ExternalInput")
    rmat_i = nc.dram_tensor("rmat", [128, 128], dt.float16, kind="ExternalInput")
    tmask = nc.dram_tensor("tmask", [128, 896], dt.float16, kind="ExternalInput")
    logitsT = nc.dram_tensor("logitsT", [VL, S], dt.float32, kind="ExternalOutput")

    with tile.TileContext(nc) as tc:
        with tc.tile_pool(name="sb", bufs=1) as sb, \
             tc.tile_pool(name="ps", bufs=1, space="PSUM") as ps, \
             tc.tile_pool(name="dr", bufs=1, space="DRAM") as dr:

            # ---- internal DRAM ----
            hdr = [dr.tile([D, S], dt.float16, tag=f"h{i}", bufs=1, name=f"h{i}")
                   for i in range(3)]           # h after sites 0..2
            arin = [[dr.tile([D, SC], dt.float16, tag=f"ai{i}{ch}",
                             bufs=1, name=f"ai{i}{ch}") for ch in range(NCH)]
                    for i in range(2 * L)]
            arout = [[dr.tile([D, SC], dt.float16, tag=f"ao{i}{ch}",
                              bufs=1, addr_space="Shared",
                              name=f"ao{i}{ch}") for ch in range(NCH)]
                     for i in range(2 * L)]
            mkTd = [dr.tile([128, HL, M], dt.float16, tag=f"mk{l}", bufs=1,
                            name=f"mk{l}") for l in range(L)]
            mvd = [dr.tile([128, HL, NMT, DH], dt.float16, tag=f"mv{l}", bufs=1,
                           name=f"mv{l}") for l in range(L)]

            # ---- global constants in SBUF ----
            kc = sb.tile([128, 2, T], dt.float16, tag="kc", bufs=1, name="kc")
            nc.sync.dma_start(kc[:], kcs[:])
            rmat = sb.tile([128, 128], dt.float16, tag="rm", bufs=1, name="rmat")
            nc.sync.dma_start(rmat[:], rmat_i[:])
            mask = sb.tile([128, 896], dt.float16, tag="msk", bufs=1, name="mask")
            nc.sync.dma_start(mask[:], tmask[:])
            ones_bf = sb.tile([128, 1], dt.float16, tag="o1", bufs=1, name="ones_bf")
            nc.vector.memset(ones_bf[:], 1.0)
            ones_row = sb.tile([1, 128], dt.float16, tag="o2", bufs=1, name="ones_row")
            nc.vector.memset(ones_row[:], 1.0)
            inv64_row = sb.tile([1, 128], dt.float16, tag="o3", bufs=1, name="inv64_row")
            nc.vector.memset(inv64_row[:], 1.0 / 64.0)

            def mm_ps(name):
                return ps.tile([128, 512], dt.float32, tag="mm", bufs=5, name=name)

            def aux_ps(name, rows=1):
                return ps.tile([rows, 512], dt.float32, tag="aux", bufs=1,
                               name=name)

            def evf(name):
                return sb.tile([128, 512], dt.float32, tag="evf", bufs=1, name=name)

            def evh(name):
                return sb.tile([128, 512], dt.float16, tag="evh", bufs=2, name=name)

            def rope_apply(raw_ps, cos_ap, sin_ap, out_ap, w=512):
                """raw_ps: [128,w] psum f32 (pre-rope head tile, d on part).
                Writes rope'd fp16 into out_ap."""
                raw_bf = sb.tile([128, 512], dt.float16, tag="rraw", bufs=2,
                                 name="raw_bf")
                nc.vector.tensor_copy(raw_bf[:, :w], raw_ps)
                r_ps = mm_ps("r_ps")
                nc.tensor.matmul(r_ps[:, :w], rmat[:], raw_bf[:, :w],
                                 start=True, stop=True)
                m1 = sb.tile([128, 512], dt.float16, tag="rt", bufs=2, name="m1")
                nc.vector.tensor_tensor(m1[:, :w], raw_bf[:, :w], cos_ap, ALU.mult)
                m2 = sb.tile([128, 512], dt.float16, tag="rt2", bufs=2, name="m2")
                nc.vector.tensor_tensor(m2[:, :w], r_ps[:, :w], sin_ap, ALU.mult)
                nc.vector.tensor_tensor(out_ap, m1[:, :w], m2[:, :w], ALU.add)

            # =========================================================
            # memory K/V projections for one layer -> DRAM
            # (memory loaded in two M-halves of 256 to save SBUF)
            # =========================================================
            def memkv(l):
                for mh in range(2):
                    msl = slice(256 * mh, 256 * (mh + 1))
                    mem_h = sb.tile([128, C, 256], dt.float16, tag="xm", bufs=1,
                                    name=f"mem{l}{mh}")
                    nc.sync.dma_start(mem_h[:], memT[l, mh])
                    for d in range(4):
                        wmk = sb.tile([128, C, 128], dt.float16, tag="wp", bufs=4,
                                      name=f"wmk{l}{mh}{d}")
                        nc.sync.dma_start(wmk[:], wmT[l, 0, d])
                        acc = mm_ps(f"mk{l}{mh}{d}")
                        for c in range(C):
                            nc.tensor.matmul(
                                acc[:, :256], wmk[:, c, :],
                                mem_h[:, c, :], start=(c == 0),
                                stop=(c == C - 1))
                        mko = sb.tile([128, 512], dt.float16, tag="pt", bufs=3,
                                      name="mko")
                        rope_apply(acc[:, :256], kc[:, 0, msl], kc[:, 1, msl],
                                   mko[:, :256], w=256)
                        nc.sync.dma_start(mkTd[l][:, d, msl], mko[:, :256])
                    for dv in range(4):
                        wmv = sb.tile([128, C, 128], dt.float16, tag="wp", bufs=4,
                                      name=f"wmv{l}{mh}{dv}")
                        nc.sync.dma_start(wmv[:], wmT[l, 1, dv])
                        for mti in range(2):
                            mt = 2 * mh + mti
                            acc = mm_ps(f"mv{l}{mh}{dv}{mt}")
                            for c in range(C):
                                nc.tensor.matmul(
                                    acc[:, :128],
                                    mem_h[:, c, 128 * mti:128 * (mti + 1)],
                                    wmv[:, c, :], start=(c == 0),
                                    stop=(c == C - 1))
                            mvo = sb.tile([128, 1, 128], dt.float16, tag="mvo",
                                          bufs=3, name="mvo")
                            nc.vector.tensor_copy(mvo[:, 0, :], acc[:, :128])
                            nc.sync.dma_start(mvd[l][:, dv:dv + 1, mt, :],
                                              mvo[:])

            # =========================================================
            # rms for ONE chunk: xb = rms(h_src + delta); h_dst written.
            # Squares split gpsimd/vector; sum-of-squares via one matmul.
            # =========================================================
            def rms_chunk(h_src, delta, h_dst, xb, ch, name):
                """h_src: DRAM [D,S] AP; delta: DRAM [D,SC] AP or None;
                h_dst: DRAM [D,S] AP or None; xb: SBUF tile [128,C,SC]."""
                hv = h_src.rearrange("(c p) s -> p c s", p=128)
                ssl = slice(SC * ch, SC * (ch + 1))
                sqacc = sb.tile([128, 512], dt.float16, tag="sqa", bufs=2,
                                name=f"sqa_{name}")
                for cq in range(C // 2):
                    csl = slice(2 * cq, 2 * cq + 2)
                    nc.sync.dma_start(xb[:, csl, :], hv[:, csl, ssl])
                    if delta is not None:
                        dtl = sb.tile([128, 2, 512], dt.float16, tag="dl",
                                      bufs=2, name="dtl")
                        nc.sync.dma_start(
                            dtl[:],
                            delta.rearrange("(c p) s -> p c s", p=128)[:, csl, :])
                        for ci in range(2):
                            nc.vector.tensor_tensor(xb[:, 2 * cq + ci, :],
                                                    xb[:, 2 * cq + ci, :],
                                                    dtl[:, ci, :], ALU.add)
                        if h_dst is not None:
                            nc.sync.dma_start(
                                h_dst.rearrange("(c p) s -> p c s", p=128)
                                [:, csl, ssl], xb[:, csl, :])
                    for ci in range(2):
                        c = 2 * cq + ci
                        hsq = sb.tile([128, 512], dt.float16, tag="hsq",
                                      bufs=2, name=f"hsq_{name}")
                        eng = nc.gpsimd if (c % 2 == 0) else nc.vector
                        eng.tensor_tensor(hsq[:], xb[:, c, :],
                                          xb[:, c, :], ALU.mult)
                        if c == 0:
                            nc.vector.tensor_copy(sqacc[:], hsq[:])
                        else:
                            nc.vector.tensor_tensor(sqacc[:], sqacc[:],
                                                    hsq[:], ALU.add)
                ssq = aux_ps(f"ssq_{name}")
                nc.tensor.matmul(ssq[:1, :], ones_bf[:], sqacc[:], start=True,
                                 stop=True)
                sq = sb.tile([1, 512], dt.float32, tag="row", bufs=1, name="sq")
                nc.scalar.activation(sq[:], ssq[:1, :], AF.Sqrt, bias=EPS,
                                     scale=1.0 / D)
                rs = sb.tile([1, 512], dt.float16, tag="row2", bufs=1, name="rs")
                with nc.allow_low_precision(reason="fp16 row for broadcast mm"):
                    nc.vector.reciprocal(rs[:], sq[:])
                bc = ps.tile([128, 512], dt.float32, tag="bc", bufs=2, name="bc")
                nc.tensor.matmul(bc[:], ones_row[:], rs[:], start=True, stop=True)
                bcs = sb.tile([128, 512], dt.float16, tag="bcs", bufs=2,
                              name=f"bcs_{name}")
                nc.vector.tensor_copy(bcs[:], bc[:])
                for c in range(C):
                    nc.vector.tensor_tensor(xb[:, c, :], xb[:, c, :], bcs[:],
                                            ALU.mult)

            # =========================================================
            # attention pieces (per layer, per chunk)
            # =========================================================
            def kv_chunk(l, ch, xb, KT, Vt):
                for d in range(4):
                    wk = sb.tile([128, C, 128], dt.float16, tag="wp", bufs=4,
                                 name=f"wk{l}{ch}{d}")
                    nc.sync.dma_start(wk[:], wqkvT[l, 1, d])
                    acc = mm_ps(f"k{l}{d}{ch}")
                    for c in range(C):
                        nc.tensor.matmul(
                            acc[:], wk[:, c, :],
                            xb[:, c, :], start=(c == 0), stop=(c == C - 1))
                    rope_apply(acc, kc[:, 0, M + SC * ch:M + SC * (ch + 1)],
                               kc[:, 1, M + SC * ch:M + SC * (ch + 1)],
                               KT[:, d, M + SC * ch:M + SC * (ch + 1)])
                for dv in range(4):
                    wv = sb.tile([128, C, 128], dt.float16, tag="wp", bufs=4,
                                 name=f"wv{l}{ch}{dv}")
                    nc.sync.dma_start(wv[:], wqkvT[l, 2, dv])
                    for sti in range(4):
                        st = 4 * ch + sti
                        acc = mm_ps(f"v{l}{dv}{st}")
                        for c in range(C):
                            nc.tensor.matmul(
                                acc[:, :128],
                                xb[:, c, 128 * sti:128 * (sti + 1)],
                                wv[:, c, :], start=(c == 0), stop=(c == C - 1))
                        nc.vector.tensor_copy(
                            Vt[:, dv, NMT + st, :], acc[:, :128])

            def q_chunk(l, ch, xb, qTc):
                """qTc: [128, HL, 512] rope'd queries for this chunk."""
                for h in range(HL):
                    wqh = sb.tile([128, C, 128], dt.float16, tag="wp", bufs=4,
                                  name=f"wq{l}{ch}{h}")
                    nc.sync.dma_start(wqh[:], wqkvT[l, 0, h])
                    acc = mm_ps(f"q{l}{h}{ch}")
                    for c in range(C):
                        nc.tensor.matmul(
                            acc[:], wqh[:, c, :],
                            xb[:, c, :], start=(c == 0), stop=(c == C - 1))
                    rope_apply(acc, kc[:, 0, M + SC * ch:M + SC * (ch + 1)],
                               kc[:, 1, M + SC * ch:M + SC * (ch + 1)],
                               qTc[:, h, :])

            def scores_chunk(l, sb_i, qTc, KT, Vt, oTc):
                """softmax(qK)V for query chunk sb_i; writes oTc [128,HL,512]."""
                ntt = NMT + 4 * (sb_i + 1)
                for h in range(HL):
                    o_ps = mm_ps(f"o{l}{h}{sb_i}")
                    s_ps = aux_ps(f"s{l}{h}{sb_i}")
                    for tt in range(ntt):
                        sc_ps = mm_ps(f"sc{l}{h}{sb_i}{tt}")
                        nc.tensor.matmul(sc_ps[:],
                                         KT[:, h, 128 * tt:128 * (tt + 1)],
                                         qTc[:, h, :], start=True, stop=True)
                        dtile = tt - ntt + 4      # >= 0 -> diagonal tile
                        if dtile >= 0:
                            off = 384 - 128 * dtile
                            nc.vector.tensor_tensor(
                                sc_ps[:], sc_ps[:],
                                mask[:, off:off + 512], ALU.add)
                        pt = sb.tile([128, 512], dt.float16, tag="pt",
                                     bufs=3, name="pt")
                        nc.scalar.activation(pt[:], sc_ps[:], AF.Exp,
                                             scale=SCALE)
                        nc.tensor.matmul(o_ps[:], Vt[:, h, tt, :], pt[:],
                                         start=(tt == 0), stop=(tt == ntt - 1))
                        nc.tensor.matmul(s_ps[:], ones_bf[:], pt[:],
                                         start=(tt == 0), stop=(tt == ntt - 1))
                    rrf = sb.tile([1, 512], dt.float32, tag="rowf", bufs=1,
                                  name="rrf")
                    nc.vector.reciprocal(rrf[:], s_ps[:])
                    rr = sb.tile([1, 512], dt.float16, tag="row2", bufs=1,
                                 name="rr")
                    with nc.allow_low_precision(reason="fp16 row for broadcast mm"):
                        nc.vector.tensor_scalar_mul(rr[:], rrf[:], 64.0)
                    bc = ps.tile([128, 512], dt.float32, tag="bc", bufs=2,
                                 name="bca")
                    nc.tensor.matmul(bc[:], inv64_row[:], rr[:],
                                     start=True, stop=True)
                    bcs = sb.tile([128, 512], dt.float32, tag="bcs2",
                                  bufs=1, name="bcs2")
                    nc.vector.tensor_copy(bcs[:], bc[:])
                    nc.vector.tensor_tensor(oTc[:, h, :], o_ps[:], bcs[:],
                                            ALU.mult)

            def wo_chunk(l, ch, oTc, site):
                for qo in range(4):
                    wo = sb.tile([128, HL, 1024], dt.float16, tag="wp", bufs=4,
                                 name=f"wo{l}{ch}{qo}")
                    nc.sync.dma_start(wo[:], woT[l, qo])
                    for do in range(8):
                        acc = mm_ps(f"wo{l}{qo}{do}{ch}")
                        for hh in range(HL):
                            nc.tensor.matmul(
                                acc[:], wo[:, hh, 128 * do:128 * (do + 1)],
                                oTc[:, hh, :],
                                start=(hh == 0), stop=(hh == HL - 1))
                        ev = evh("woev")
                        nc.vector.tensor_copy(ev[:], acc[:])
                        nc.sync.dma_start(
                            arin[site][ch]
                            .rearrange("(c p) s -> p c s", p=128)
                            [:, 8 * qo + do, :], ev[:])
                nc.gpsimd.collective_compute(
                    "AllReduce", ALU.add,
                    replica_groups=[list(range(NCORES))],
                    ins=[arin[site][ch][:]],
                    outs=[arout[site][ch][:]])

            # =========================================================
            # FFN for one chunk
            # =========================================================
            def ffn_chunk(l, ch, xb, site):
                actT = sb.tile([128, FL // 128, 512], dt.float16, tag="actT",
                               bufs=1, name=f"actT{l}{ch}")
                for fe in range(FL // 128):
                    wg = sb.tile([128, C, 128], dt.float16, tag="wp", bufs=4,
                                 name=f"wg{l}{ch}{fe}")
                    nc.sync.dma_start(wg[:], wguT[l, 0, fe])
                    gs = sb.tile([128, 512], dt.float16, tag="gs", bufs=2,
                                 name="gs")
                    acc = mm_ps(f"g{l}{fe}{ch}")
                    for c in range(C):
                        nc.tensor.matmul(acc[:], wg[:, c, :], xb[:, c, :],
                                         start=(c == 0), stop=(c == C - 1))
                    nc.scalar.activation(gs[:], acc[:], AF.Silu)
                    wu = sb.tile([128, C, 128], dt.float16, tag="wp", bufs=4,
                                 name=f"wu{l}{ch}{fe}")
                    nc.sync.dma_start(wu[:], wguT[l, 1, fe])
                    acc2 = mm_ps(f"u{l}{fe}{ch}")
                    for c in range(C):
                        nc.tensor.matmul(acc2[:], wu[:, c, :], xb[:, c, :],
                                         start=(c == 0), stop=(c == C - 1))
                    nc.vector.tensor_tensor(actT[:, fe, :], acc2[:], gs[:],
                                            ALU.mult)
                for e in range(8):
                    wd = sb.tile([128, FL // 128, 512], dt.float16, tag="wp",
                                 bufs=4, name=f"wd{l}{ch}{e}")
                    nc.sync.dma_start(wd[:], wdT[l, e])
                    for do in range(4):
                        acc = mm_ps(f"wd{l}{e}{do}{ch}")
                        for fc in range(FL // 128):
                            nc.tensor.matmul(
                                acc[:], wd[:, fc, 128 * do:128 * (do + 1)],
                                actT[:, fc, :],
                                start=(fc == 0), stop=(fc == FL // 128 - 1))
                        ev = evh("wdev")
                        nc.vector.tensor_copy(ev[:], acc[:])
                        nc.sync.dma_start(
                            arin[site][ch]
                            .rearrange("(c p) s -> p c s", p=128)
                            [:, 4 * e + do, :], ev[:])
                nc.gpsimd.collective_compute(
                    "AllReduce", ALU.add,
                    replica_groups=[list(range(NCORES))],
                    ins=[arin[site][ch][:]],
                    outs=[arout[site][ch][:]])

            # =========================================================
            # LM head for one chunk
            # =========================================================
            def lm_chunk(ch, xb):
                for v8 in range(8):
                    lm = sb.tile([128, C, 128], dt.float16, tag="wp", bufs=4,
                                 name=f"lm{ch}{v8}")
                    nc.sync.dma_start(lm[:], lmT[v8])
                    acc = mm_ps(f"lm{v8}{ch}")
                    for c in range(C):
                        nc.tensor.matmul(
                            acc[:], lm[:, c, :],
                            xb[:, c, :], start=(c == 0), stop=(c == C - 1))
                    ev = evf("lmev")
                    nc.vector.tensor_copy(ev[:], acc[:])
                    nc.sync.dma_start(
                        logitsT[:].rearrange("(t p) s -> p t s", p=128)
                        [:, v8, SC * ch:SC * (ch + 1)], ev[:])

            # =========================================================
            # main flow
            # =========================================================
            def xb_tile(nm):
                return sb.tile([128, C, SC], dt.float16, tag="xb", bufs=2,
                               name=nm)

            def kvt_tiles(l):
                KT = sb.tile([128, HL, T], dt.float16, tag="KT", bufs=1,
                             name=f"KT{l}")
                Vt = sb.tile([128, HL, NTT, DH], dt.float16, tag="V", bufs=1,
                             name=f"V{l}")
                nc.sync.dma_start(KT[:, :, :M], mkTd[l][:])
                nc.sync.dma_start(Vt[:, :, :NMT, :], mvd[l][:])
                return KT, Vt

            def qo_tiles(l, ch):
                qTc = sb.tile([128, HL, 512], dt.float16, tag="qT", bufs=2,
                              name=f"qT{l}{ch}")
                oTc = sb.tile([128, HL, 512], dt.float16, tag="oT", bufs=2,
                              name=f"oT{l}{ch}")
                return qTc, oTc

            # ---- prephase: memory K/V both layers, initial rms ----
            memkv(0)
            memkv(1)
            x_a0 = [xb_tile(f"xa0{ch}") for ch in range(NCH)]
            rms_chunk(h0T[:], None, None, x_a0[0], 0, "ra0")
            rms_chunk(h0T[:], None, None, x_a0[1], 1, "ra1")

            # ---- attention layer 0 (site 0) ----
            KT0, Vt0 = kvt_tiles(0)
            q00, o00 = qo_tiles(0, 0)
            kv_chunk(0, 0, x_a0[0], KT0, Vt0)
            q_chunk(0, 0, x_a0[0], q00)
            scores_chunk(0, 0, q00, KT0, Vt0, o00)
            wo_chunk(0, 0, o00, 0)
            q01, o01 = qo_tiles(0, 1)
            kv_chunk(0, 1, x_a0[1], KT0, Vt0)
            q_chunk(0, 1, x_a0[1], q01)
            scores_chunk(0, 1, q01, KT0, Vt0, o01)
            wo_chunk(0, 1, o01, 0)

            # ---- FFN layer 0 (site 1) ----
            x_f0 = [xb_tile(f"xf0{ch}") for ch in range(NCH)]
            rms_chunk(h0T[:], arout[0][0][:], hdr[0][:], x_f0[0], 0, "rf00")
            ffn_chunk(0, 0, x_f0[0], 1)
            rms_chunk(h0T[:], arout[0][1][:], hdr[0][:], x_f0[1], 1, "rf01")
            ffn_chunk(0, 1, x_f0[1], 1)

            # ---- attention layer 1 (site 2) ----
            x_a1 = [xb_tile(f"xa1{ch}") for ch in range(NCH)]
            rms_chunk(hdr[0][:], arout[1][0][:], hdr[1][:], x_a1[0], 0, "ra10")
            KT1, Vt1 = kvt_tiles(1)
            q10, o10 = qo_tiles(1, 0)
            kv_chunk(1, 0, x_a1[0], KT1, Vt1)
            q_chunk(1, 0, x_a1[0], q10)
            scores_chunk(1, 0, q10, KT1, Vt1, o10)
            wo_chunk(1, 0, o10, 2)
            rms_chunk(hdr[0][:], arout[1][1][:], hdr[1][:], x_a1[1], 1, "ra11")
            q11, o11 = qo_tiles(1, 1)
            kv_chunk(1, 1, x_a1[1], KT1, Vt1)
            q_chunk(1, 1, x_a1[1], q11)
            scores_chunk(1, 1, q11, KT1, Vt1, o11)
            wo_chunk(1, 1, o11, 2)

            # ---- FFN layer 1 (site 3) ----
            x_f1 = [xb_tile(f"xf1{ch}") for ch in range(NCH)]
            rms_chunk(hdr[1][:], arout[2][0][:], hdr[2][:], x_f1[0], 0, "rf10")
            ffn_chunk(1, 0, x_f1[0], 3)
            rms_chunk(hdr[1][:], arout[2][1][:], hdr[2][:], x_f1[1], 1, "rf11")
            ffn_chunk(1, 1, x_f1[1], 3)

            # ---- final rms + LM head ----
            xf = [xb_tile(f"xl{ch}") for ch in range(NCH)]
            rms_chunk(hdr[2][:], arout[3][0][:], None, xf[0], 0, "rl0")
            lm_chunk(0, xf[0])
            rms_chunk(hdr[2][:], arout[3][1][:], None, xf[1], 1, "rl1")
            lm_chunk(1, xf[1])

    nc.finalize()
    return nc


_NC_CACHE = {}


def _get_module():
    if "nc" not in _NC_CACHE:
        _NC_CACHE["nc"] = build_module()
    return _NC_CACHE["nc"]


def _rope_tables():
    inv_freq = 1.0 / (ROPE_BASE ** (np.arange(0, DH, 2, dtype=np.float64) / DH))
    ang = np.arange(T, dtype=np.float64)[:, None] * inv_freq[None, :]
    emb = np.concatenate([ang, ang], axis=-1)          # [T, DH]
    return np.cos(emb).astype(np.float32), np.sin(emb).astype(np.float32)


def kernel(input_ids, memory, embed, Wq, Wk, Wv, Wo, Wg, Wu, Wd, Wmk, Wmv,
           ln1, ln2, normw, lm_head):
    input_ids = np.asarray(input_ids)
    f32 = np.float32
    memory = np.asarray(memory, f32)

    nc = _get_module()

    # host prep: embedding gather (pure data movement) + layout transforms
    h0 = np.asarray(embed, f32)[input_ids.reshape(-1)]          # [S, D]
    h0T = np.ascontiguousarray(h0.T).astype(BF)                 # [D, S] fp16

    cos, sin = _rope_tables()
    qcs = np.stack([cos[M:], sin[M:]]).transpose(2, 0, 1)       # [128, 2, S]
    kcs = np.stack([cos, sin]).transpose(2, 0, 1)               # [128, 2, T]

    rmat = np.zeros((128, 128), f32)
    for d in range(64):
        rmat[d + 64, d] = -1.0
        rmat[d, d + 64] = 1.0

    tmaskv = np.full((128, 896), NEG, f32)
    for t in range(128):
        tmaskv[t, 384 + t:] = 0.0

    def bf(x):
        return np.ascontiguousarray(x).astype(BF)

    def swz(wT, nsplit):
        """[Din, n] (Din = c*128) -> [nsplit, 128, c, n/nsplit]."""
        c = wT.shape[0] // 128
        n = wT.shape[1]
        w = wT.reshape(c, 128, n).transpose(1, 0, 2)            # [128, c, n]
        w = w.reshape(128, c, nsplit, n // nsplit).transpose(2, 0, 1, 3)
        return w

    memT = np.stack([swz(memory[l, 0].T, 2) for l in range(L)])  # [L,2,128,C,256]

    # fold RMSNorm weights into the GEMM weights (applied along d-in)
    ln1 = np.asarray(ln1, f32)
    ln2 = np.asarray(ln2, f32)
    normw = np.asarray(normw, f32)
    Wq_f = np.asarray(Wq, f32) * ln1[:, None, :]
    Wk_f = np.asarray(Wk, f32) * ln1[:, None, :]
    Wv_f = np.asarray(Wv, f32) * ln1[:, None, :]
    Wg_f = np.asarray(Wg, f32) * ln2[:, None, :]
    Wu_f = np.asarray(Wu, f32) * ln2[:, None, :]
    lm_f = np.asarray(lm_head, f32) * normw[None, :]

    in_maps = []
    for i in range(NCORES):
        hs = slice(DL * i, DL * (i + 1))
        fs = slice(FL * i, FL * (i + 1))
        vs = slice(VL * i, VL * (i + 1))
        in_maps.append({
            "h0T": h0T,
            "memT": bf(memT),
            "wqkvT": bf(np.stack([np.stack([swz(W[l][hs].T, 4)
                                            for W in (Wq_f, Wk_f, Wv_f)])
                                  for l in range(L)])),
            "wmT": bf(np.stack([np.stack([swz(np.asarray(W, f32)[l][hs].T, 4)
                                          for W in (Wmk, Wmv)])
                                for l in range(L)])),
            "woT": bf(np.stack([swz(np.asarray(Wo, f32)[l][:, hs].T, 4)
                                for l in range(L)])),
            "wguT": bf(np.stack([np.stack([swz(W[l][fs].T, 8)
                                           for W in (Wg_f, Wu_f)])
                                 for l in range(L)])),
            "wdT": bf(np.stack([swz(np.asarray(Wd, f32)[l][:, fs].T, 8)
                                for l in range(L)])),
            "lmT": bf(swz(lm_f[vs].T, 8)),
            "kcs": bf(kcs),
            "rmat": bf(rmat),
            "tmask": bf(tmaskv),
        })

    res = run_bass_kernel_spmd(nc, in_maps, core_ids=list(range(NCORES)))
    _NC_CACHE["last_results"] = res

    logits = np.empty((B, S, V), f32)
    for i in range(NCORES):
        logits[0, :, VL * i:VL * (i + 1)] = res.results[i]["logitsT"].T
    return logits
